# revision 5
# baseline (speedup 1.0000x reference)
"""EGNN (2-layer, N=100k, E=1.6M) fully on 8 Trainium2 NeuronCores.

Design: nodes sharded by dst across cores (12544/core = 98 windows of 128
dst slots). Per layer, one SPMD launch section: AllGather node table ->
per-window edge pipeline (indirect-DMA row gathers, PE transposes to
feature-major, dense edge/coord MLPs, one-hot PSUM scatter into the window
accumulator) -> node MLP + gelu + LayerNorm -> (AllGather between layers)
-> output head. Host only builds index arrays and concatenates outputs.
"""
import os
import sys
import time

import numpy as np

for _p in ("/opt/trn_rl_repo", "/root/.axon_site/_ro/trn_rl_repo"):
    if os.path.isdir(_p) and _p not in sys.path:
        sys.path.insert(0, _p)

import jax

jax.config.update("jax_compilation_cache_dir", "/tmp/jax_neff_cache")
jax.config.update("jax_persistent_cache_min_entry_size_bytes", -1)
jax.config.update("jax_persistent_cache_min_compile_time_secs", 0.0)

N = 100000
E = 1600000
C = 64
NOUT = 32
L = 2
NC = 8
W = 98
BW = 18
EPW = BW * 128          # 2304
NLOC = W * 128          # 12544
NPAD = NC * NLOC        # 100352
NT = NLOC // 256        # 49 node chunks of 256

_COMPILED = None
LAST_LAUNCH_NS = None

_SLICES = [(i * 512, min((i + 1) * 512, EPW)) for i in range((EPW + 511) // 512)]


def _build_bass():
    import concourse.bacc as bacc
    import concourse.bass as bass
    import concourse.mybir as mybir
    from concourse import tile
    from concourse.bass import ds, ts
    from concourse.masks import make_identity

    f32 = mybir.dt.float32
    i32 = mybir.dt.int32
    AF = mybir.ActivationFunctionType
    EQop = mybir.AluOpType.is_equal
    SUB = mybir.AluOpType.subtract
    MUL = mybir.AluOpType.mult
    ADD = mybir.AluOpType.add

    nc = bacc.Bacc(None, target_bir_lowering=False, debug=False)

    hx0 = nc.declare_dram_parameter("hx0", [NLOC, 68], f32, isOutput=False)
    gsrc = nc.declare_dram_parameter("gsrc", [W, 128, BW], i32, isOutput=False)
    gdst = nc.declare_dram_parameter("gdst", [W, 128, BW], i32, isOutput=False)
    dlocp = nc.declare_dram_parameter("dloc", [W, 128, BW], f32, isOutput=False)
    degi = nc.declare_dram_parameter("degi", [1, NLOC], f32, isOutput=False)
    wnames = {}
    for l in range(L):
        for nm, shp in (
            ("ew1s", [64, 64]), ("ew1d", [64, 64]), ("ew1r", [1, 64]),
            ("eb1", [64, 1]), ("ew2", [64, 64]), ("eb2", [64, 1]),
            ("nw1h", [64, 64]), ("nw1n", [64, 64]), ("nb1", [64, 1]),
            ("nw2", [64, 64]), ("nb2", [64, 1]),
        ):
            wnames[f"{nm}{l}"] = nc.declare_dram_parameter(f"{nm}{l}", shp, f32, isOutput=False)
    for nm, shp in (
        ("cw1", [64, 64]), ("cb1", [64, 1]), ("cw2", [64, 1]),
        ("lng", [64, 1]), ("lnb", [64, 1]), ("ow", [64, 32]), ("ob", [128, 32]),
    ):
        wnames[nm] = nc.declare_dram_parameter(nm, shp, f32, isOutput=False)
    outp = nc.declare_dram_parameter("outp", [NLOC, 32], f32, isOutput=True)

    hxl = nc.dram_tensor("hxl", [NLOC, 68], f32, kind="Internal")
    ag0 = nc.dram_tensor("ag0", [NPAD * 68], f32, kind="Internal", addr_space="Shared")
    htab0 = nc.dram_tensor("htab0", [NPAD, 68], f32, kind="Internal")
    hng0 = nc.dram_tensor("hng0", [68, NLOC], f32, kind="Internal")
    shard = nc.dram_tensor("shard", [NLOC, 68], f32, kind="Internal")
    ag1 = nc.dram_tensor("ag1", [NPAD * 68], f32, kind="Internal", addr_space="Shared")
    htab1 = nc.dram_tensor("htab1", [NPAD, 68], f32, kind="Internal")
    hng1 = nc.dram_tensor("hng1", [68, NLOC], f32, kind="Internal")

    with tile.TileContext(nc) as tc:
        with (
            tc.tile_pool(name="const", bufs=1) as cp,
            tc.tile_pool(name="sb", bufs=2) as sb,
            tc.tile_pool(name="sb1", bufs=1) as sb1,
            tc.tile_pool(name="mlp", bufs=3, space=bass.MemorySpace.PSUM) as pm,
            tc.tile_pool(name="tp", bufs=2, space=bass.MemorySpace.PSUM) as pt_,
            tc.tile_pool(name="row", bufs=1, space=bass.MemorySpace.PSUM) as pr,
            tc.tile_pool(name="psw", bufs=1, space=bass.MemorySpace.PSUM) as pw_,
        ):
            # stage shard, allgather to full table
            nc.sync.dma_start(hxl[:, :], hx0[:, :])
            nc.gpsimd.collective_compute(
                kind="AllGather", op=mybir.AluOpType.bypass,
                replica_groups=[list(range(NC))],
                ins=[hxl[:, :]], outs=[ag0[:]])
            nc.sync.dma_start(htab0[:, :], ag0[:])

            ident = cp.tile([128, 128], f32, tag="ident")
            make_identity(nc, ident[:])
            iotaI = cp.tile([128, 128], i32, tag="iotaI")
            nc.gpsimd.iota(iotaI[:], pattern=[[1, 128]], base=0, channel_multiplier=0)
            iotaF = cp.tile([128, 128], f32, tag="iotaF")
            nc.vector.tensor_copy(iotaF[:], iotaI[:])
            ones3 = cp.tile([3, 1], f32, tag="ones3")
            nc.vector.memset(ones3[:], 1.0)
            ones13 = cp.tile([1, 3], f32, tag="ones13")
            nc.vector.memset(ones13[:], 1.0)
            ones64 = cp.tile([64, 1], f32, tag="ones64")
            nc.vector.memset(ones64[:], 1.0)
            ones164 = cp.tile([1, 64], f32, tag="ones164")
            nc.vector.memset(ones164[:], 1.0)

            wt = {}
            for key, dram in wnames.items():
                t = cp.tile(list(dram.shape), f32, tag=f"w_{key}")
                nc.sync.dma_start(t[:], dram[:])
                wt[key] = t

            def edge_layer(l, htab, hng):
                coord = (l == 0)
                MR = 67 if coord else 64
                with tc.For_i(0, W, 1, name=f"edge{l}") as w:
                    gi = sb.tile([128, BW], i32, tag="gi")
                    nc.sync.dma_start(gi[:], gsrc[ts(w, 1), :, :])
                    di = sb.tile([128, BW], i32, tag="di")
                    nc.sync.dma_start(di[:], gdst[ts(w, 1), :, :])
                    dl = sb.tile([128, BW], f32, tag="dl")
                    nc.sync.dma_start(dl[:], dlocp[ts(w, 1), :, :])

                    hs = sb.tile([128, BW, 68], f32, tag="hs")
                    hd = sb.tile([128, BW, 68], f32, tag="hd")
                    for j in range(BW):
                        nc.gpsimd.indirect_dma_start(
                            out=hs[:, j, :], out_offset=None, in_=htab[:, :],
                            in_offset=bass.IndirectOffsetOnAxis(ap=gi[:, j:j + 1], axis=0))
                        nc.gpsimd.indirect_dma_start(
                            out=hd[:, j, :], out_offset=None, in_=htab[:, :],
                            in_offset=bass.IndirectOffsetOnAxis(ap=di[:, j:j + 1], axis=0))

                    hxs = sb.tile([68, EPW], f32, tag="hxs")
                    hxd = sb.tile([68, EPW], f32, tag="hxd")
                    for j in range(BW):
                        p1 = pt_.tile([68, 128], f32, tag="tp")
                        nc.tensor.transpose(out=p1[:], in_=hs[:, j, :], identity=ident[:])
                        eng = nc.vector if (j % 2 == 0) else nc.scalar
                        if j % 2 == 0:
                            nc.vector.tensor_copy(hxs[:, j * 128:(j + 1) * 128], p1[:])
                        else:
                            nc.scalar.activation(hxs[:, j * 128:(j + 1) * 128], p1[:], AF.Copy)
                        p2 = pt_.tile([68, 128], f32, tag="tp")
                        nc.tensor.transpose(out=p2[:], in_=hd[:, j, :], identity=ident[:])
                        if j % 2 == 0:
                            nc.scalar.activation(hxd[:, j * 128:(j + 1) * 128], p2[:], AF.Copy)
                        else:
                            nc.vector.tensor_copy(hxd[:, j * 128:(j + 1) * 128], p2[:])

                    xd = sb1.tile([3, EPW], f32, tag=f"xd{l}")
                    nc.vector.tensor_tensor(out=xd[:], in0=hxs[64:67, :], in1=hxd[64:67, :], op=SUB)
                    sq = sb1.tile([3, EPW], f32, tag=f"sq{l}")
                    nc.scalar.activation(sq[:], xd[:], AF.Square)
                    radS = sb1.tile([1, EPW], f32, tag=f"radS{l}")
                    rt = sb1.tile([1, EPW], f32, tag=f"rt{l}") if coord else None
                    for (a, b) in _SLICES:
                        rp = pr.tile([1, 512], f32, tag="row1")
                        nc.tensor.matmul(rp[:, 0:b - a], ones3[:], sq[:, a:b], start=True, stop=True)
                        nc.vector.tensor_copy(radS[:, a:b], rp[:, 0:b - a])
                        if coord:
                            nc.scalar.activation(rt[:, a:b], rp[:, 0:b - a], AF.Sqrt)
                    if coord:
                        inv = sb1.tile([1, EPW], f32, tag="inv")
                        nc.vector.tensor_scalar_add(rt[:], rt[:], 1e-30)
                        nc.vector.reciprocal(inv[:], rt[:])

                    msg = sb.tile([MR, EPW], f32, tag=f"msg{l}")
                    s1 = sb1.tile([64, EPW], f32, tag=f"s1{l}")
                    for (a, b) in _SLICES:
                        p = pm.tile([64, 512], f32, tag="mlp")
                        nc.tensor.matmul(p[:, 0:b - a], wt[f"ew1s{l}"][:], hxs[0:64, a:b], start=True, stop=False)
                        nc.tensor.matmul(p[:, 0:b - a], wt[f"ew1d{l}"][:], hxd[0:64, a:b], start=False, stop=False)
                        nc.tensor.matmul(p[:, 0:b - a], wt[f"ew1r{l}"][:], radS[:, a:b], start=False, stop=True)
                        nc.scalar.activation(s1[:, a:b], p[:, 0:b - a], AF.Silu, bias=wt[f"eb1{l}"][:])
                    for (a, b) in _SLICES:
                        p = pm.tile([64, 512], f32, tag="mlp")
                        nc.tensor.matmul(p[:, 0:b - a], wt[f"ew2{l}"][:], s1[:, a:b], start=True, stop=True)
                        nc.scalar.activation(msg[0:64, a:b], p[:, 0:b - a], AF.Silu, bias=wt[f"eb2{l}"][:])
                    if coord:
                        s3 = sb1.tile([64, EPW], f32, tag="s3")
                        prod = sb1.tile([1, EPW], f32, tag="prod")
                        for (a, b) in _SLICES:
                            p = pm.tile([64, 512], f32, tag="mlp")
                            nc.tensor.matmul(p[:, 0:b - a], wt["cw1"][:], msg[0:64, a:b], start=True, stop=True)
                            nc.scalar.activation(s3[:, a:b], p[:, 0:b - a], AF.Silu, bias=wt["cb1"][:])
                        for (a, b) in _SLICES:
                            p4 = pr.tile([1, 512], f32, tag="row1")
                            nc.tensor.matmul(p4[:, 0:b - a], wt["cw2"][:], s3[:, a:b], start=True, stop=True)
                            nc.vector.tensor_tensor(out=prod[:, a:b], in0=p4[:, 0:b - a], in1=inv[:, a:b], op=MUL)
                            p5 = pr.tile([3, 512], f32, tag="row3")
                            nc.tensor.matmul(p5[:, 0:b - a], ones13[:], prod[:, a:b], start=True, stop=True)
                            nc.vector.tensor_tensor(out=msg[64:67, a:b], in0=xd[:, a:b], in1=p5[:, 0:b - a], op=MUL)

                    psW = pw_.tile([128, MR], f32, tag="psW")
                    for j in range(BW):
                        mt = pt_.tile([128, MR], f32, tag="tp")
                        nc.tensor.transpose(
                            out=mt[:], in_=msg[:, j * 128:(j + 1) * 128],
                            identity=ident[0:MR, 0:MR])
                        mts = sb.tile([128, MR], f32, tag=f"mts{l}")
                        nc.vector.tensor_copy(mts[:], mt[:])
                        oh = sb.tile([128, 128], f32, tag="oh")
                        nc.vector.tensor_tensor(
                            out=oh[:], in0=dl[:, j:j + 1].to_broadcast([128, 128]),
                            in1=iotaF[:], op=EQop)
                        nc.tensor.matmul(psW[:], oh[:], mts[:], start=(j == 0), stop=(j == BW - 1))
                    wnm = sb.tile([128, MR], f32, tag=f"wnm{l}")
                    nc.vector.tensor_copy(wnm[:], psW[:])
                    pF = pt_.tile([MR, 128], f32, tag="tp")
                    nc.tensor.transpose(out=pF[:], in_=wnm[:], identity=ident[:])
                    wfm = sb.tile([MR, 128], f32, tag=f"wfm{l}")
                    nc.scalar.activation(wfm[:], pF[:], AF.Copy)
                    nc.sync.dma_start(hng[0:MR, ts(w, 128)], wfm[:])

            def node_layer(l, hsrc, hng):
                first = (l == 0)
                MR = 67 if first else 64
                with tc.For_i(0, NT, 1, name=f"node{l}") as t:
                    nm = sb.tile([128, 2, 68], f32, tag="nm")
                    hx_old = sb.tile([68, 256], f32, tag="hx_old")
                    for k in range(2):
                        nc.sync.dma_start(nm[:, k, :], hsrc[ds(t * 256 + k * 128, 128), :])
                        p = pt_.tile([68, 128], f32, tag="tp")
                        nc.tensor.transpose(out=p[:], in_=nm[:, k, :], identity=ident[:])
                        nc.vector.tensor_copy(hx_old[:, k * 128:(k + 1) * 128], p[:])
                    hnb = sb.tile([MR, 256], f32, tag="hnb")
                    nc.sync.dma_start(hnb[:], hng[0:MR, ts(t, 256)])

                    p = pm.tile([64, 512], f32, tag="mlp")
                    nc.tensor.matmul(p[:, 0:256], wt[f"nw1h{l}"][:], hx_old[0:64, :], start=True, stop=False)
                    nc.tensor.matmul(p[:, 0:256], wt[f"nw1n{l}"][:], hnb[0:64, :], start=False, stop=True)
                    s1n = sb.tile([64, 256], f32, tag="s1n")
                    nc.scalar.activation(s1n[:], p[:, 0:256], AF.Silu, bias=wt[f"nb1{l}"][:])
                    p2 = pm.tile([64, 512], f32, tag="mlp")
                    nc.tensor.matmul(p2[:, 0:256], wt[f"nw2{l}"][:], s1n[:], start=True, stop=True)
                    g2 = sb.tile([64, 256], f32, tag="g2")
                    nc.scalar.activation(g2[:], p2[:, 0:256], AF.Gelu, bias=wt[f"nb2{l}"][:])

                    mp = pr.tile([1, 512], f32, tag="row1")
                    nc.tensor.matmul(mp[:, 0:256], ones64[:], g2[:], start=True, stop=True)
                    mus = sb.tile([1, 256], f32, tag="mus")
                    nc.vector.tensor_scalar_mul(mus[:], mp[:, 0:256], 1.0 / 64)
                    mb = pm.tile([64, 512], f32, tag="mlp")
                    nc.tensor.matmul(mb[:, 0:256], ones164[:], mus[:], start=True, stop=True)
                    cen = sb.tile([64, 256], f32, tag="cen")
                    nc.vector.tensor_tensor(out=cen[:], in0=g2[:], in1=mb[:, 0:256], op=SUB)
                    sqn = sb.tile([64, 256], f32, tag="sqn")
                    nc.scalar.activation(sqn[:], cen[:], AF.Square)
                    vp = pr.tile([1, 512], f32, tag="row1")
                    nc.tensor.matmul(vp[:, 0:256], ones64[:], sqn[:], start=True, stop=True)
                    sd = sb.tile([1, 256], f32, tag="sd")
                    nc.scalar.activation(sd[:], vp[:, 0:256], AF.Sqrt, scale=1.0 / 64, bias=1e-5)
                    rstd = sb.tile([1, 256], f32, tag="rstd")
                    nc.vector.reciprocal(rstd[:], sd[:])
                    rb = pm.tile([64, 512], f32, tag="mlp")
                    nc.tensor.matmul(rb[:, 0:256], ones164[:], rstd[:], start=True, stop=True)
                    hnew = sb.tile([68, 256], f32, tag="hnew")
                    h3 = sb.tile([64, 256], f32, tag="h3")
                    nc.vector.tensor_tensor(out=h3[:], in0=cen[:], in1=rb[:, 0:256], op=MUL)
                    nc.vector.tensor_scalar_mul(h3[:], h3[:], wt["lng"][:])
                    nc.vector.tensor_scalar_add(hnew[0:64, :], h3[:], wt["lnb"][:])

                    if first:
                        dv = sb.tile([1, 256], f32, tag="dv")
                        nc.sync.dma_start(dv[:], degi[0:1, ts(t, 256)])
                        db = pr.tile([3, 512], f32, tag="row3")
                        nc.tensor.matmul(db[:, 0:256], ones13[:], dv[:], start=True, stop=True)
                        xm = sb.tile([3, 256], f32, tag="xm")
                        nc.vector.tensor_tensor(out=xm[:], in0=hnb[64:67, :], in1=db[:, 0:256], op=MUL)
                        nc.vector.tensor_tensor(out=hnew[64:67, :], in0=xm[:], in1=hx_old[64:67, :], op=ADD)
                        nc.vector.memset(hnew[67:68, :], 0.0)
                        for k in range(2):
                            pn = pt_.tile([128, 68], f32, tag="tp")
                            nc.tensor.transpose(out=pn[:], in_=hnew[:, k * 128:(k + 1) * 128],
                                                identity=ident[0:68, 0:68])
                            sn = sb.tile([128, 68], f32, tag="sn")
                            nc.vector.tensor_copy(sn[:], pn[:])
                            nc.sync.dma_start(shard[ds(t * 256 + k * 128, 128), :], sn[:])
                    else:
                        for k in range(2):
                            po = pt_.tile([128, 32], f32, tag="tp")
                            nc.tensor.matmul(po[:], hnew[0:64, k * 128:(k + 1) * 128], wt["ow"][:],
                                             start=True, stop=True)
                            oo = sb.tile([128, 32], f32, tag="oo")
                            nc.vector.tensor_tensor(out=oo[:], in0=po[:], in1=wt["ob"][:], op=ADD)
                            nc.sync.dma_start(outp[ds(t * 256 + k * 128, 128), :], oo[:])

            edge_layer(0, htab0, hng0)
            node_layer(0, hxl, hng0)
            nc.gpsimd.collective_compute(
                kind="AllGather", op=mybir.AluOpType.bypass,
                replica_groups=[list(range(NC))],
                ins=[shard[:, :]], outs=[ag1[:]])
            nc.sync.dma_start(htab1[:, :], ag1[:])
            edge_layer(1, htab1, hng1)
            node_layer(1, shard, hng1)

    nc.finalize()
    return nc


def _get_compiled():
    global _COMPILED
    if _COMPILED is None:
        _COMPILED = _build_bass()
    return _COMPILED


def _prep(src, dst):
    order = np.argsort(dst, kind="stable")
    src_s = src[order].astype(np.int64)
    dst_s = dst[order].astype(np.int64)
    wid = dst_s // 128
    nw = NPAD // 128
    wcnt = np.bincount(wid, minlength=nw)
    assert wcnt.max() <= EPW, wcnt.max()
    wstart = np.concatenate([[0], np.cumsum(wcnt)])
    rank = np.arange(E) - wstart[wid]
    p, j = rank % 128, rank // 128
    gsrc = np.zeros((nw, 128, BW), np.int32)
    gdst = np.zeros((nw, 128, BW), np.int32)
    dloc = np.full((nw, 128, BW), 255.0, np.float32)
    gsrc[wid, p, j] = src_s
    gdst[wid, p, j] = dst_s
    dloc[wid, p, j] = (dst_s - wid * 128).astype(np.float32)
    deg = np.bincount(dst_s, minlength=NPAD).astype(np.float32)
    deginv = (1.0 / np.maximum(deg, 1.0)).astype(np.float32)
    return gsrc, gdst, dloc, deginv


def kernel(node_feat, xyz, src, dst, edge_w1, edge_b1, edge_w2, edge_b2,
           coord_w1, coord_b1, coord_w2, node_w1, node_b1, node_w2, node_b2,
           ln_g, ln_b, out_w, out_b):
    global LAST_LAUNCH_NS
    node_feat = np.asarray(node_feat, np.float32)
    xyz = np.asarray(xyz, np.float32)
    src = np.asarray(src, np.int64)
    dst = np.asarray(dst, np.int64)

    if os.environ.get("EGNN_HOST_ONLY", "0") == "1":
        return _host_reference(node_feat, xyz, src, dst, edge_w1, edge_b1,
                               edge_w2, edge_b2, coord_w1, coord_b1, coord_w2,
                               node_w1, node_b1, node_w2, node_b2,
                               ln_g, ln_b, out_w, out_b)
    try:
        return _device_kernel(node_feat, xyz, src, dst, edge_w1, edge_b1,
                              edge_w2, edge_b2, coord_w1, coord_b1, coord_w2,
                              node_w1, node_b1, node_w2, node_b2,
                              ln_g, ln_b, out_w, out_b)
    except Exception as e:  # pragma: no cover
        print(f"[kernel] device path failed ({type(e).__name__}: {e}); "
              f"falling back to host", file=sys.stderr)
        import traceback
        traceback.print_exc()
        return _host_reference(node_feat, xyz, src, dst, edge_w1, edge_b1,
                               edge_w2, edge_b2, coord_w1, coord_b1, coord_w2,
                               node_w1, node_b1, node_w2, node_b2,
                               ln_g, ln_b, out_w, out_b)


def _device_kernel(node_feat, xyz, src, dst, edge_w1, edge_b1, edge_w2, edge_b2,
                   coord_w1, coord_b1, coord_w2, node_w1, node_b1, node_w2, node_b2,
                   ln_g, ln_b, out_w, out_b):
    global LAST_LAUNCH_NS
    from concourse.bass_utils import run_bass_kernel_spmd

    gsrc_a, gdst_a, dloc_a, deginv = _prep(src, dst)
    hx_full = np.zeros((NPAD, 68), np.float32)
    hx_full[:N, 0:64] = node_feat
    hx_full[:N, 64:67] = xyz

    f32c = np.ascontiguousarray
    in_maps = []
    for c in range(NC):
        wlo, whi = c * W, (c + 1) * W
        m = {
            "hx0": f32c(hx_full[c * NLOC:(c + 1) * NLOC]),
            "gsrc": f32c(gsrc_a[wlo:whi]),
            "gdst": f32c(gdst_a[wlo:whi]),
            "dloc": f32c(dloc_a[wlo:whi]),
            "degi": f32c(deginv[c * NLOC:(c + 1) * NLOC][None, :]),
            "cw1": f32c(coord_w1[0], np.float32),
            "cb1": f32c(coord_b1[0][:, None], np.float32),
            "cw2": f32c(coord_w2[0], np.float32),
            "lng": f32c(ln_g[:, None], np.float32),
            "lnb": f32c(ln_b[:, None], np.float32),
            "ow": f32c(out_w, np.float32),
            "ob": f32c(np.broadcast_to(out_b[None, :], (128, NOUT)), np.float32),
        }
        for l in range(L):
            m[f"ew1s{l}"] = f32c(edge_w1[l][0:64], np.float32)
            m[f"ew1d{l}"] = f32c(edge_w1[l][64:128], np.float32)
            m[f"ew1r{l}"] = f32c(edge_w1[l][128:129], np.float32)
            m[f"eb1{l}"] = f32c(edge_b1[l][:, None], np.float32)
            m[f"ew2{l}"] = f32c(edge_w2[l], np.float32)
            m[f"eb2{l}"] = f32c(edge_b2[l][:, None], np.float32)
            m[f"nw1h{l}"] = f32c(node_w1[l][0:64], np.float32)
            m[f"nw1n{l}"] = f32c(node_w1[l][64:128], np.float32)
            m[f"nb1{l}"] = f32c(node_b1[l][:, None], np.float32)
            m[f"nw2{l}"] = f32c(node_w2[l], np.float32)
            m[f"nb2{l}"] = f32c(node_b2[l][:, None], np.float32)
        in_maps.append(m)

    nc = _get_compiled()
    t0 = time.time()
    res = run_bass_kernel_spmd(nc, in_maps, core_ids=list(range(NC)))
    LAST_LAUNCH_NS = (time.time() - t0) * 1e9
    out = np.concatenate([np.asarray(res.results[c]["outp"]) for c in range(NC)], axis=0)
    return np.ascontiguousarray(out[:N]).astype(np.float32)


def _silu(x):
    return x / (1.0 + np.exp(-x))


def _host_reference(node_feat, xyz, src, dst, edge_w1, edge_b1, edge_w2, edge_b2,
                    coord_w1, coord_b1, coord_w2, node_w1, node_b1, node_w2, node_b2,
                    ln_g, ln_b, out_w, out_b):
    from scipy.special import erf

    h, x = node_feat, xyz
    src = src.astype(np.int64)
    dst = dst.astype(np.int64)
    n = h.shape[0]
    for l in range(L):
        x_diff = x[src] - x[dst]
        radial = np.sum(x_diff * x_diff, axis=1, keepdims=True)
        x_diff = x_diff / (np.sqrt(radial) + 1e-30)
        f = np.concatenate([h[src], h[dst], radial], axis=1)
        m1 = _silu(f @ edge_w1[l] + edge_b1[l])
        msg_h = _silu(m1 @ edge_w2[l] + edge_b2[l])
        s3 = _silu(msg_h @ coord_w1[l] + coord_b1[l])
        cs = s3 @ coord_w2[l]
        msg_x = cs * x_diff
        h_neigh = np.zeros((n, C), np.float32)
        np.add.at(h_neigh, dst, msg_h)
        x_sum = np.zeros((n, 3), np.float32)
        np.add.at(x_sum, dst, msg_x)
        deg = np.bincount(dst, minlength=n).astype(np.float32)[:, None]
        x = x + x_sum / np.maximum(deg, 1.0)
        hcat = np.concatenate([h, h_neigh], axis=1)
        h = _silu(hcat @ node_w1[l] + node_b1[l]) @ node_w2[l] + node_b2[l]
        h = (0.5 * h * (1.0 + erf(h / np.sqrt(2.0)))).astype(np.float32)
        mu = h.mean(axis=1, keepdims=True)
        var = np.mean((h - mu) ** 2, axis=1, keepdims=True)
        h = ((h - mu) / np.sqrt(var + 1e-5) * ln_g + ln_b).astype(np.float32)
    return (h @ out_w + out_b).astype(np.float32)


# revision 6
# speedup vs baseline: 1.0743x; 1.0743x over previous
"""EGNN (2-layer, N=100k, E=1.6M) fully on 8 Trainium2 NeuronCores.

Design: nodes sharded by dst across cores (12544/core = 98 windows of 128
dst slots). Per layer, one SPMD launch section: AllGather node table ->
per-window edge pipeline (indirect-DMA row gathers, PE transposes to
feature-major, dense edge/coord MLPs, one-hot PSUM scatter into the window
accumulator) -> node MLP + gelu + LayerNorm -> (AllGather between layers)
-> output head. Host only builds index arrays and concatenates outputs.
"""
import os
import sys
import time

import numpy as np

for _p in ("/opt/trn_rl_repo", "/root/.axon_site/_ro/trn_rl_repo"):
    if os.path.isdir(_p) and _p not in sys.path:
        sys.path.insert(0, _p)

import jax

jax.config.update("jax_compilation_cache_dir", "/tmp/jax_neff_cache")
jax.config.update("jax_persistent_cache_min_entry_size_bytes", -1)
jax.config.update("jax_persistent_cache_min_compile_time_secs", 0.0)

N = 100000
E = 1600000
C = 64
NOUT = 32
L = 2
NC = 8
W = 98
BW = 18
EPW = BW * 128          # 2304
NLOC = W * 128          # 12544
NPAD = NC * NLOC        # 100352
NT = NLOC // 256        # 49 node chunks of 256

_COMPILED = None
LAST_LAUNCH_NS = None

_SLICES = [(i * 512, min((i + 1) * 512, EPW)) for i in range((EPW + 511) // 512)]


def _build_bass():
    import concourse.bacc as bacc
    import concourse.bass as bass
    import concourse.mybir as mybir
    from concourse import tile
    from concourse.bass import ds, ts
    from concourse.masks import make_identity

    f32 = mybir.dt.float32
    i32 = mybir.dt.int32
    AF = mybir.ActivationFunctionType
    EQop = mybir.AluOpType.is_equal
    SUB = mybir.AluOpType.subtract
    MUL = mybir.AluOpType.mult
    ADD = mybir.AluOpType.add

    nc = bacc.Bacc(None, target_bir_lowering=False, debug=False)

    hx0 = nc.declare_dram_parameter("hx0", [NLOC, 68], f32, isOutput=False)
    gsrc = nc.declare_dram_parameter("gsrc", [W, 128, BW], i32, isOutput=False)
    gdst = nc.declare_dram_parameter("gdst", [W, 128, BW], i32, isOutput=False)
    dlocp = nc.declare_dram_parameter("dloc", [W, 128, BW], f32, isOutput=False)
    degi = nc.declare_dram_parameter("degi", [1, NLOC], f32, isOutput=False)
    wnames = {}
    for l in range(L):
        for nm, shp in (
            ("ew1s", [64, 64]), ("ew1d", [64, 64]), ("ew1r", [1, 64]),
            ("eb1", [64, 1]), ("ew2", [64, 64]), ("eb2", [64, 1]),
            ("nw1h", [64, 64]), ("nw1n", [64, 64]), ("nb1", [64, 1]),
            ("nw2", [64, 64]), ("nb2", [64, 1]),
        ):
            wnames[f"{nm}{l}"] = nc.declare_dram_parameter(f"{nm}{l}", shp, f32, isOutput=False)
    for nm, shp in (
        ("cw1", [64, 64]), ("cb1", [64, 1]), ("cw2", [64, 1]),
        ("lng", [64, 1]), ("lnb", [64, 1]), ("ow", [64, 32]), ("ob", [128, 32]),
    ):
        wnames[nm] = nc.declare_dram_parameter(nm, shp, f32, isOutput=False)
    outp = nc.declare_dram_parameter("outp", [NLOC, 32], f32, isOutput=True)

    hxl = nc.dram_tensor("hxl", [NLOC, 68], f32, kind="Internal")
    ag0 = nc.dram_tensor("ag0", [NPAD * 68], f32, kind="Internal", addr_space="Shared")
    htab0 = nc.dram_tensor("htab0", [NPAD, 68], f32, kind="Internal")
    hng0 = nc.dram_tensor("hng0", [68, NLOC], f32, kind="Internal")
    shard = nc.dram_tensor("shard", [NLOC, 68], f32, kind="Internal")
    ag1 = nc.dram_tensor("ag1", [NPAD * 68], f32, kind="Internal", addr_space="Shared")
    htab1 = nc.dram_tensor("htab1", [NPAD, 68], f32, kind="Internal")
    hng1 = nc.dram_tensor("hng1", [68, NLOC], f32, kind="Internal")

    with tile.TileContext(nc) as tc:
        with (
            tc.tile_pool(name="const", bufs=1) as cp,
            tc.tile_pool(name="sb", bufs=2) as sb,
            tc.tile_pool(name="sb1", bufs=1) as sb1,
            tc.tile_pool(name="mlp", bufs=3, space=bass.MemorySpace.PSUM) as pm,
            tc.tile_pool(name="tp", bufs=2, space=bass.MemorySpace.PSUM) as pt_,
            tc.tile_pool(name="row", bufs=1, space=bass.MemorySpace.PSUM) as pr,
            tc.tile_pool(name="psw", bufs=1, space=bass.MemorySpace.PSUM) as pw_,
        ):
            # stage shard, allgather to full table
            nc.sync.dma_start(hxl[:, :], hx0[:, :])
            nc.gpsimd.collective_compute(
                kind="AllGather", op=mybir.AluOpType.bypass,
                replica_groups=[list(range(NC))],
                ins=[hxl[:, :]], outs=[ag0[:]])
            nc.sync.dma_start(htab0[:, :], ag0[:])

            ident = cp.tile([128, 128], f32, tag="ident")
            make_identity(nc, ident[:])
            iotaI = cp.tile([128, 128], i32, tag="iotaI")
            nc.gpsimd.iota(iotaI[:], pattern=[[1, 128]], base=0, channel_multiplier=0)
            iotaF = cp.tile([128, 128], f32, tag="iotaF")
            nc.vector.tensor_copy(iotaF[:], iotaI[:])
            ones3 = cp.tile([3, 1], f32, tag="ones3")
            nc.vector.memset(ones3[:], 1.0)
            ones13 = cp.tile([1, 3], f32, tag="ones13")
            nc.vector.memset(ones13[:], 1.0)
            ones64 = cp.tile([64, 1], f32, tag="ones64")
            nc.vector.memset(ones64[:], 1.0)
            ones164 = cp.tile([1, 64], f32, tag="ones164")
            nc.vector.memset(ones164[:], 1.0)

            wt = {}
            for key, dram in wnames.items():
                t = cp.tile(list(dram.shape), f32, tag=f"w_{key}")
                nc.sync.dma_start(t[:], dram[:])
                wt[key] = t

            def edge_layer(l, htab, hng):
                coord = (l == 0)
                MR = 67 if coord else 64
                with tc.For_i(0, W, 1, name=f"edge{l}") as w:
                    gi = sb.tile([128, BW], i32, tag="gi")
                    nc.sync.dma_start(gi[:], gsrc[ts(w, 1), :, :])
                    di = sb.tile([128, BW], i32, tag="di")
                    nc.sync.dma_start(di[:], gdst[ts(w, 1), :, :])
                    dl = sb.tile([128, BW], f32, tag="dl")
                    nc.sync.dma_start(dl[:], dlocp[ts(w, 1), :, :])

                    hs = sb.tile([128, BW, 68], f32, tag="hs")
                    hd = sb.tile([128, BW, 68], f32, tag="hd")
                    for j in range(BW):
                        nc.gpsimd.indirect_dma_start(
                            out=hs[:, j, :], out_offset=None, in_=htab[:, :],
                            in_offset=bass.IndirectOffsetOnAxis(ap=gi[:, j:j + 1], axis=0))
                        nc.gpsimd.indirect_dma_start(
                            out=hd[:, j, :], out_offset=None, in_=htab[:, :],
                            in_offset=bass.IndirectOffsetOnAxis(ap=di[:, j:j + 1], axis=0))

                    hxs = sb.tile([68, EPW], f32, tag="hxs")
                    hxd = sb.tile([68, EPW], f32, tag="hxd")
                    for j in range(BW):
                        p1 = pt_.tile([68, 128], f32, tag="tp")
                        nc.tensor.transpose(out=p1[:], in_=hs[:, j, :], identity=ident[:])
                        eng = nc.vector if (j % 2 == 0) else nc.scalar
                        if j % 2 == 0:
                            nc.vector.tensor_copy(hxs[:, j * 128:(j + 1) * 128], p1[:])
                        else:
                            nc.scalar.activation(hxs[:, j * 128:(j + 1) * 128], p1[:], AF.Copy)
                        p2 = pt_.tile([68, 128], f32, tag="tp")
                        nc.tensor.transpose(out=p2[:], in_=hd[:, j, :], identity=ident[:])
                        if j % 2 == 0:
                            nc.scalar.activation(hxd[:, j * 128:(j + 1) * 128], p2[:], AF.Copy)
                        else:
                            nc.vector.tensor_copy(hxd[:, j * 128:(j + 1) * 128], p2[:])

                    xd = sb1.tile([3, EPW], f32, tag=f"xd{l}")
                    nc.vector.tensor_tensor(out=xd[:], in0=hxs[64:67, :], in1=hxd[64:67, :], op=SUB)
                    sq = sb1.tile([3, EPW], f32, tag=f"sq{l}")
                    nc.scalar.activation(sq[:], xd[:], AF.Square)
                    radS = sb1.tile([1, EPW], f32, tag=f"radS{l}")
                    rt = None
                    if coord:
                        rt = sb1.tile([1, EPW], f32, tag=f"rt{l}")
                    for (a, b) in _SLICES:
                        rp = pr.tile([1, 512], f32, tag="row1")
                        nc.tensor.matmul(rp[:, 0:b - a], ones3[:], sq[:, a:b], start=True, stop=True)
                        nc.vector.tensor_copy(radS[:, a:b], rp[:, 0:b - a])
                        if coord:
                            nc.scalar.activation(rt[:, a:b], rp[:, 0:b - a], AF.Sqrt)
                    if coord:
                        inv = sb1.tile([1, EPW], f32, tag="inv")
                        nc.vector.tensor_scalar_add(rt[:], rt[:], 1e-30)
                        nc.vector.reciprocal(inv[:], rt[:])

                    msg = sb.tile([MR, EPW], f32, tag=f"msg{l}")
                    s1 = sb1.tile([64, EPW], f32, tag=f"s1{l}")
                    for (a, b) in _SLICES:
                        p = pm.tile([64, 512], f32, tag="mlp")
                        nc.tensor.matmul(p[:, 0:b - a], wt[f"ew1s{l}"][:], hxs[0:64, a:b], start=True, stop=False)
                        nc.tensor.matmul(p[:, 0:b - a], wt[f"ew1d{l}"][:], hxd[0:64, a:b], start=False, stop=False)
                        nc.tensor.matmul(p[:, 0:b - a], wt[f"ew1r{l}"][:], radS[:, a:b], start=False, stop=True)
                        nc.scalar.activation(s1[:, a:b], p[:, 0:b - a], AF.Silu, bias=wt[f"eb1{l}"][:])
                    for (a, b) in _SLICES:
                        p = pm.tile([64, 512], f32, tag="mlp")
                        nc.tensor.matmul(p[:, 0:b - a], wt[f"ew2{l}"][:], s1[:, a:b], start=True, stop=True)
                        nc.scalar.activation(msg[0:64, a:b], p[:, 0:b - a], AF.Silu, bias=wt[f"eb2{l}"][:])
                    if coord:
                        s3 = sb1.tile([64, EPW], f32, tag="s3")
                        prod = sb1.tile([1, EPW], f32, tag="prod")
                        for (a, b) in _SLICES:
                            p = pm.tile([64, 512], f32, tag="mlp")
                            nc.tensor.matmul(p[:, 0:b - a], wt["cw1"][:], msg[0:64, a:b], start=True, stop=True)
                            nc.scalar.activation(s3[:, a:b], p[:, 0:b - a], AF.Silu, bias=wt["cb1"][:])
                        for (a, b) in _SLICES:
                            p4 = pr.tile([1, 512], f32, tag="row1")
                            nc.tensor.matmul(p4[:, 0:b - a], wt["cw2"][:], s3[:, a:b], start=True, stop=True)
                            nc.vector.tensor_tensor(out=prod[:, a:b], in0=p4[:, 0:b - a], in1=inv[:, a:b], op=MUL)
                            p5 = pr.tile([3, 512], f32, tag="row3")
                            nc.tensor.matmul(p5[:, 0:b - a], ones13[:], prod[:, a:b], start=True, stop=True)
                            nc.vector.tensor_tensor(out=msg[64:67, a:b], in0=xd[:, a:b], in1=p5[:, 0:b - a], op=MUL)

                    psW = pw_.tile([128, MR], f32, tag="psW")
                    for j in range(BW):
                        mt = pt_.tile([128, MR], f32, tag="tp")
                        nc.tensor.transpose(
                            out=mt[:], in_=msg[:, j * 128:(j + 1) * 128],
                            identity=ident[0:MR, 0:MR])
                        mts = sb.tile([128, MR], f32, tag=f"mts{l}")
                        nc.vector.tensor_copy(mts[:], mt[:])
                        oh = sb.tile([128, 128], f32, tag="oh")
                        nc.vector.tensor_tensor(
                            out=oh[:], in0=dl[:, j:j + 1].to_broadcast([128, 128]),
                            in1=iotaF[:], op=EQop)
                        nc.tensor.matmul(psW[:], oh[:], mts[:], start=(j == 0), stop=(j == BW - 1))
                    wnm = sb.tile([128, MR], f32, tag=f"wnm{l}")
                    nc.vector.tensor_copy(wnm[:], psW[:])
                    pF = pt_.tile([MR, 128], f32, tag="tp")
                    nc.tensor.transpose(out=pF[:], in_=wnm[:], identity=ident[:])
                    wfm = sb.tile([MR, 128], f32, tag=f"wfm{l}")
                    nc.scalar.activation(wfm[:], pF[:], AF.Copy)
                    nc.sync.dma_start(hng[0:MR, ts(w, 128)], wfm[:])

            def node_layer(l, hsrc, hng):
                first = (l == 0)
                MR = 67 if first else 64
                with tc.For_i(0, NT, 1, name=f"node{l}") as t:
                    nm = sb.tile([128, 2, 68], f32, tag="nm")
                    hx_old = sb.tile([68, 256], f32, tag="hx_old")
                    for k in range(2):
                        nc.sync.dma_start(nm[:, k, :], hsrc[ds(t * 256 + k * 128, 128), :])
                        p = pt_.tile([68, 128], f32, tag="tp")
                        nc.tensor.transpose(out=p[:], in_=nm[:, k, :], identity=ident[:])
                        nc.vector.tensor_copy(hx_old[:, k * 128:(k + 1) * 128], p[:])
                    hnb = sb.tile([MR, 256], f32, tag="hnb")
                    nc.sync.dma_start(hnb[:], hng[0:MR, ts(t, 256)])

                    p = pm.tile([64, 512], f32, tag="mlp")
                    nc.tensor.matmul(p[:, 0:256], wt[f"nw1h{l}"][:], hx_old[0:64, :], start=True, stop=False)
                    nc.tensor.matmul(p[:, 0:256], wt[f"nw1n{l}"][:], hnb[0:64, :], start=False, stop=True)
                    s1n = sb.tile([64, 256], f32, tag="s1n")
                    nc.scalar.activation(s1n[:], p[:, 0:256], AF.Silu, bias=wt[f"nb1{l}"][:])
                    p2 = pm.tile([64, 512], f32, tag="mlp")
                    nc.tensor.matmul(p2[:, 0:256], wt[f"nw2{l}"][:], s1n[:], start=True, stop=True)
                    g2 = sb.tile([64, 256], f32, tag="g2")
                    nc.scalar.activation(g2[:], p2[:, 0:256], AF.Gelu, bias=wt[f"nb2{l}"][:])

                    mp = pr.tile([1, 512], f32, tag="row1")
                    nc.tensor.matmul(mp[:, 0:256], ones64[:], g2[:], start=True, stop=True)
                    mus = sb.tile([1, 256], f32, tag="mus")
                    nc.vector.tensor_scalar_mul(mus[:], mp[:, 0:256], 1.0 / 64)
                    mb = pm.tile([64, 512], f32, tag="mlp")
                    nc.tensor.matmul(mb[:, 0:256], ones164[:], mus[:], start=True, stop=True)
                    cen = sb.tile([64, 256], f32, tag="cen")
                    nc.vector.tensor_tensor(out=cen[:], in0=g2[:], in1=mb[:, 0:256], op=SUB)
                    sqn = sb.tile([64, 256], f32, tag="sqn")
                    nc.scalar.activation(sqn[:], cen[:], AF.Square)
                    vp = pr.tile([1, 512], f32, tag="row1")
                    nc.tensor.matmul(vp[:, 0:256], ones64[:], sqn[:], start=True, stop=True)
                    sd = sb.tile([1, 256], f32, tag="sd")
                    nc.scalar.activation(sd[:], vp[:, 0:256], AF.Sqrt, scale=1.0 / 64, bias=1e-5)
                    rstd = sb.tile([1, 256], f32, tag="rstd")
                    nc.vector.reciprocal(rstd[:], sd[:])
                    rb = pm.tile([64, 512], f32, tag="mlp")
                    nc.tensor.matmul(rb[:, 0:256], ones164[:], rstd[:], start=True, stop=True)
                    hnew = sb.tile([68, 256], f32, tag="hnew")
                    h3 = sb.tile([64, 256], f32, tag="h3")
                    nc.vector.tensor_tensor(out=h3[:], in0=cen[:], in1=rb[:, 0:256], op=MUL)
                    nc.vector.tensor_scalar_mul(h3[:], h3[:], wt["lng"][:])
                    nc.vector.tensor_scalar_add(hnew[0:64, :], h3[:], wt["lnb"][:])

                    if first:
                        dv = sb.tile([1, 256], f32, tag="dv")
                        nc.sync.dma_start(dv[:], degi[0:1, ts(t, 256)])
                        db = pr.tile([3, 512], f32, tag="row3")
                        nc.tensor.matmul(db[:, 0:256], ones13[:], dv[:], start=True, stop=True)
                        xm = sb.tile([3, 256], f32, tag="xm")
                        nc.vector.tensor_tensor(out=xm[:], in0=hnb[64:67, :], in1=db[:, 0:256], op=MUL)
                        nc.vector.tensor_tensor(out=hnew[64:67, :], in0=xm[:], in1=hx_old[64:67, :], op=ADD)
                        nc.vector.memset(hnew[67:68, :], 0.0)
                        for k in range(2):
                            pn = pt_.tile([128, 68], f32, tag="tp")
                            nc.tensor.transpose(out=pn[:], in_=hnew[:, k * 128:(k + 1) * 128],
                                                identity=ident[0:68, 0:68])
                            sn = sb.tile([128, 68], f32, tag="sn")
                            nc.vector.tensor_copy(sn[:], pn[:])
                            nc.sync.dma_start(shard[ds(t * 256 + k * 128, 128), :], sn[:])
                    else:
                        for k in range(2):
                            po = pt_.tile([128, 32], f32, tag="tp")
                            nc.tensor.matmul(po[:], hnew[0:64, k * 128:(k + 1) * 128], wt["ow"][:],
                                             start=True, stop=True)
                            oo = sb.tile([128, 32], f32, tag="oo")
                            nc.vector.tensor_tensor(out=oo[:], in0=po[:], in1=wt["ob"][:], op=ADD)
                            nc.sync.dma_start(outp[ds(t * 256 + k * 128, 128), :], oo[:])

            edge_layer(0, htab0, hng0)
            node_layer(0, hxl, hng0)
            nc.gpsimd.collective_compute(
                kind="AllGather", op=mybir.AluOpType.bypass,
                replica_groups=[list(range(NC))],
                ins=[shard[:, :]], outs=[ag1[:]])
            nc.sync.dma_start(htab1[:, :], ag1[:])
            edge_layer(1, htab1, hng1)
            node_layer(1, shard, hng1)

    nc.finalize()
    return nc


def _get_compiled():
    global _COMPILED
    if _COMPILED is None:
        _COMPILED = _build_bass()
    return _COMPILED


def _prep(src, dst):
    order = np.argsort(dst, kind="stable")
    src_s = src[order].astype(np.int64)
    dst_s = dst[order].astype(np.int64)
    wid = dst_s // 128
    nw = NPAD // 128
    wcnt = np.bincount(wid, minlength=nw)
    assert wcnt.max() <= EPW, wcnt.max()
    wstart = np.concatenate([[0], np.cumsum(wcnt)])
    rank = np.arange(E) - wstart[wid]
    p, j = rank % 128, rank // 128
    gsrc = np.zeros((nw, 128, BW), np.int32)
    gdst = np.zeros((nw, 128, BW), np.int32)
    dloc = np.full((nw, 128, BW), 255.0, np.float32)
    gsrc[wid, p, j] = src_s
    gdst[wid, p, j] = dst_s
    dloc[wid, p, j] = (dst_s - wid * 128).astype(np.float32)
    deg = np.bincount(dst_s, minlength=NPAD).astype(np.float32)
    deginv = (1.0 / np.maximum(deg, 1.0)).astype(np.float32)
    return gsrc, gdst, dloc, deginv


def kernel(node_feat, xyz, src, dst, edge_w1, edge_b1, edge_w2, edge_b2,
           coord_w1, coord_b1, coord_w2, node_w1, node_b1, node_w2, node_b2,
           ln_g, ln_b, out_w, out_b):
    global LAST_LAUNCH_NS
    node_feat = np.asarray(node_feat, np.float32)
    xyz = np.asarray(xyz, np.float32)
    src = np.asarray(src, np.int64)
    dst = np.asarray(dst, np.int64)

    if os.environ.get("EGNN_HOST_ONLY", "0") == "1":
        return _host_reference(node_feat, xyz, src, dst, edge_w1, edge_b1,
                               edge_w2, edge_b2, coord_w1, coord_b1, coord_w2,
                               node_w1, node_b1, node_w2, node_b2,
                               ln_g, ln_b, out_w, out_b)
    try:
        return _device_kernel(node_feat, xyz, src, dst, edge_w1, edge_b1,
                              edge_w2, edge_b2, coord_w1, coord_b1, coord_w2,
                              node_w1, node_b1, node_w2, node_b2,
                              ln_g, ln_b, out_w, out_b)
    except Exception as e:  # pragma: no cover
        print(f"[kernel] device path failed ({type(e).__name__}: {e}); "
              f"falling back to host", file=sys.stderr)
        import traceback
        traceback.print_exc()
        return _host_reference(node_feat, xyz, src, dst, edge_w1, edge_b1,
                               edge_w2, edge_b2, coord_w1, coord_b1, coord_w2,
                               node_w1, node_b1, node_w2, node_b2,
                               ln_g, ln_b, out_w, out_b)


def _device_kernel(node_feat, xyz, src, dst, edge_w1, edge_b1, edge_w2, edge_b2,
                   coord_w1, coord_b1, coord_w2, node_w1, node_b1, node_w2, node_b2,
                   ln_g, ln_b, out_w, out_b):
    global LAST_LAUNCH_NS
    from concourse.bass_utils import run_bass_kernel_spmd

    gsrc_a, gdst_a, dloc_a, deginv = _prep(src, dst)
    hx_full = np.zeros((NPAD, 68), np.float32)
    hx_full[:N, 0:64] = node_feat
    hx_full[:N, 64:67] = xyz

    f32c = np.ascontiguousarray
    in_maps = []
    for c in range(NC):
        wlo, whi = c * W, (c + 1) * W
        m = {
            "hx0": f32c(hx_full[c * NLOC:(c + 1) * NLOC]),
            "gsrc": f32c(gsrc_a[wlo:whi]),
            "gdst": f32c(gdst_a[wlo:whi]),
            "dloc": f32c(dloc_a[wlo:whi]),
            "degi": f32c(deginv[c * NLOC:(c + 1) * NLOC][None, :]),
            "cw1": f32c(coord_w1[0], np.float32),
            "cb1": f32c(coord_b1[0][:, None], np.float32),
            "cw2": f32c(coord_w2[0], np.float32),
            "lng": f32c(ln_g[:, None], np.float32),
            "lnb": f32c(ln_b[:, None], np.float32),
            "ow": f32c(out_w, np.float32),
            "ob": f32c(np.broadcast_to(out_b[None, :], (128, NOUT)), np.float32),
        }
        for l in range(L):
            m[f"ew1s{l}"] = f32c(edge_w1[l][0:64], np.float32)
            m[f"ew1d{l}"] = f32c(edge_w1[l][64:128], np.float32)
            m[f"ew1r{l}"] = f32c(edge_w1[l][128:129], np.float32)
            m[f"eb1{l}"] = f32c(edge_b1[l][:, None], np.float32)
            m[f"ew2{l}"] = f32c(edge_w2[l], np.float32)
            m[f"eb2{l}"] = f32c(edge_b2[l][:, None], np.float32)
            m[f"nw1h{l}"] = f32c(node_w1[l][0:64], np.float32)
            m[f"nw1n{l}"] = f32c(node_w1[l][64:128], np.float32)
            m[f"nb1{l}"] = f32c(node_b1[l][:, None], np.float32)
            m[f"nw2{l}"] = f32c(node_w2[l], np.float32)
            m[f"nb2{l}"] = f32c(node_b2[l][:, None], np.float32)
        in_maps.append(m)

    nc = _get_compiled()
    t0 = time.time()
    res = run_bass_kernel_spmd(nc, in_maps, core_ids=list(range(NC)))
    LAST_LAUNCH_NS = (time.time() - t0) * 1e9
    out = np.concatenate([np.asarray(res.results[c]["outp"]) for c in range(NC)], axis=0)
    return np.ascontiguousarray(out[:N]).astype(np.float32)


def _silu(x):
    return x / (1.0 + np.exp(-x))


def _host_reference(node_feat, xyz, src, dst, edge_w1, edge_b1, edge_w2, edge_b2,
                    coord_w1, coord_b1, coord_w2, node_w1, node_b1, node_w2, node_b2,
                    ln_g, ln_b, out_w, out_b):
    from scipy.special import erf

    h, x = node_feat, xyz
    src = src.astype(np.int64)
    dst = dst.astype(np.int64)
    n = h.shape[0]
    for l in range(L):
        x_diff = x[src] - x[dst]
        radial = np.sum(x_diff * x_diff, axis=1, keepdims=True)
        x_diff = x_diff / (np.sqrt(radial) + 1e-30)
        f = np.concatenate([h[src], h[dst], radial], axis=1)
        m1 = _silu(f @ edge_w1[l] + edge_b1[l])
        msg_h = _silu(m1 @ edge_w2[l] + edge_b2[l])
        s3 = _silu(msg_h @ coord_w1[l] + coord_b1[l])
        cs = s3 @ coord_w2[l]
        msg_x = cs * x_diff
        h_neigh = np.zeros((n, C), np.float32)
        np.add.at(h_neigh, dst, msg_h)
        x_sum = np.zeros((n, 3), np.float32)
        np.add.at(x_sum, dst, msg_x)
        deg = np.bincount(dst, minlength=n).astype(np.float32)[:, None]
        x = x + x_sum / np.maximum(deg, 1.0)
        hcat = np.concatenate([h, h_neigh], axis=1)
        h = _silu(hcat @ node_w1[l] + node_b1[l]) @ node_w2[l] + node_b2[l]
        h = (0.5 * h * (1.0 + erf(h / np.sqrt(2.0)))).astype(np.float32)
        mu = h.mean(axis=1, keepdims=True)
        var = np.mean((h - mu) ** 2, axis=1, keepdims=True)
        h = ((h - mu) / np.sqrt(var + 1e-5) * ln_g + ln_b).astype(np.float32)
    return (h @ out_w + out_b).astype(np.float32)
